# revision 1
# baseline (speedup 1.0000x reference)
"""DeepSet segment-reduce kernel for 8 Trainium2 NeuronCores.

Math (equivalent to the reference, using linearity of segment_sum):
    r      = relu(x @ W1 + b1)                      # per-node, on device
    sums_r = segment_sum(r)                         # [B, HID]
    mean_r = sums_r / max(counts, 1)                # counts via host bincount
    hid    = mean_r @ W2 + b2 * (counts > 0)        # tiny tail, on device
    out    = relu(hid @ W3 + b3) @ W4 + b4          # tiny tail, on device

Device layout: hid on partitions, nodes on the free dim.  Each core gets a
contiguous shard of nodes split into two halves packed on partition halves
(features of half A in partitions 0..63, half B in 64..127), so DMA runs at
full 128-partition width and the two K=64 matmuls run concurrently in
different PE row groups with W1 resident in both halves of the array.

Segment sums: the host reorders each half so every segment's node run is
zero-padded to a multiple of 512 columns.  Every 512-column tile then
belongs to exactly one segment, and a fused relu+accumulate (ACT
``activation`` with ``accum_out``) per tile produces exact per-tile sums S
with no boundary fixups.  S is PE-transposed and multiplied by a per-core
0/1 routing matrix A (tile -> segment, host data) to form the per-core
partial segment sums.  The host adds the 8 partial results (and removes the
pad columns' relu(b1) contribution), then a second tiny NEFF applies the
mean and the rho MLP.
"""

import os
import sys

for _p in ("/opt/trn_rl_repo",):
    if os.path.isdir(_p) and _p not in sys.path:
        sys.path.append(_p)

import numpy as np

import concourse.bass as bass
import concourse.tile as tile
from concourse import bacc, mybir
from concourse.bass_utils import run_bass_kernel_spmd

F32 = mybir.dt.float32
F32R = mybir.dt.float32r

NCORES = 8
TILE = 512
DC = 5              # tiles per DMA chunk
NSEG = 1024
ODIM = 16


def _host_prep(x, x_batch, W1, b1, ncores=NCORES):
    N, _ = x.shape
    assert N % (2 * ncores) == 0
    ch = N // (2 * ncores)
    xb = np.asarray(x_batch)

    counts = np.bincount(xb, minlength=NSEG).astype(np.float64)

    plans = []          # (core, half) -> (src, tile_seg)
    cols_needed = 0
    for c in range(ncores):
        for h in range(2):
            lo = (2 * c + h) * ch
            ids = xb[lo:lo + ch]
            uniq, starts = np.unique(ids, return_index=True)
            ends = np.append(starts[1:], ch)
            src = np.full(ch + (len(uniq) + 1) * TILE, -1, dtype=np.int64)
            tile_seg = []
            col = 0
            for k in range(len(uniq)):
                L = int(ends[k] - starts[k])
                T = -(-L // TILE)
                src[col:col + L] = np.arange(lo + starts[k], lo + ends[k])
                tile_seg += [int(uniq[k])] * T
                col += T * TILE
            plans.append((src, tile_seg, col))
            cols_needed = max(cols_needed, col)

    ct = -(-cols_needed // TILE)
    cols = ct * TILE
    sw = -(-2 * ct // 128) * 128          # S width, multiple of 128
    nch = sw // 128

    padcount = np.zeros(NSEG, dtype=np.float64)
    in_maps = []
    w1d = np.vstack([W1, W1]).astype(np.float32)
    ident = np.eye(128, dtype=np.float32)
    for c in range(ncores):
        xt = np.zeros((128, cols), dtype=np.float32)
        amat = np.zeros((sw, NSEG), dtype=np.float32)
        for h in range(2):
            src, tile_seg, col = plans[2 * c + h]
            src = src[:col]
            if len(src) < cols:
                src = np.concatenate([src, np.full(cols - len(src), -1, np.int64)])
            else:
                src = src[:cols]
            mask = src >= 0
            gath = np.zeros((cols, 64), dtype=np.float32)
            gath[mask] = x[src[mask]]
            xt[64 * h:64 * h + 64, :] = gath.T
            for t, seg in enumerate(tile_seg):
                amat[h * ct + t, seg] = 1.0
            if tile_seg:
                seg_arr = np.array(tile_seg, dtype=np.int64)
                real = (src[:col] >= 0).reshape(-1, TILE).sum(axis=1)
                np.add.at(padcount, seg_arr, TILE - real[:len(seg_arr)])
        in_maps.append(dict(xt=xt, w1d=w1d, ident=ident, amat=amat,
                            b1=np.ascontiguousarray(b1, np.float32).reshape(128, 1)))

    meta = dict(ct=ct, cols=cols, sw=sw, nch=nch, ncores=ncores,
                counts=counts, padcount=padcount)
    return in_maps, meta


def _build_phase1(meta):
    ct, cols, sw, nch = meta["ct"], meta["cols"], meta["sw"], meta["nch"]
    ncores = meta["ncores"]

    nc = bacc.Bacc("TRN2", target_bir_lowering=False, debug=False,
                   num_devices=ncores)
    xt_d = nc.dram_tensor("xt", [128, cols], F32R, kind="ExternalInput").ap()
    w1_d = nc.dram_tensor("w1d", [128, 128], F32R, kind="ExternalInput").ap()
    b1_d = nc.dram_tensor("b1", [128, 1], F32, kind="ExternalInput").ap()
    id_d = nc.dram_tensor("ident", [128, 128], F32, kind="ExternalInput").ap()
    am_d = nc.dram_tensor("amat", [sw, NSEG], F32, kind="ExternalInput").ap()
    ps_d = nc.dram_tensor("psums", [128, NSEG], F32, kind="ExternalOutput").ap()

    with tile.TileContext(nc) as tc:
        with tc.tile_pool(name="const", bufs=1) as cpool, \
             tc.tile_pool(name="xin", bufs=3) as xpool, \
             tc.tile_pool(name="tr", bufs=4) as trpool, \
             tc.tile_pool(name="ps", bufs=5, space="PSUM") as pspool, \
             tc.tile_pool(name="psc", bufs=1, space="PSUM") as pscpool:

            w1t = cpool.tile([128, 128], F32R)
            nc.sync.dma_start(w1t[:], w1_d[:])
            b1t = cpool.tile([128, 1], F32)
            nc.sync.dma_start(b1t[:], b1_d[:])
            ident = cpool.tile([128, 128], F32)
            nc.sync.dma_start(ident[:], id_d[:])
            amat = cpool.tile([128, nch, NSEG], F32)
            nc.sync.dma_start(amat[:], am_d.rearrange("(k p) s -> p k s", p=128))
            S = cpool.tile([128, sw], F32)
            nc.vector.memset(S[:], 0.0)

            xtile = None
            for t in range(ct):
                if t % DC == 0:
                    w = min(DC, ct - t) * TILE
                    xtile = xpool.tile([128, DC * TILE], F32R, tag="xt")
                    nc.sync.dma_start(xtile[:, :w], xt_d[:, t * TILE:t * TILE + w])
                off = (t % DC) * TILE
                for half in range(2):
                    ps = pspool.tile([128, TILE], F32)
                    nc.tensor.matmul(
                        ps[:],
                        lhsT=w1t[64 * half:64 * half + 64, :],
                        rhs=xtile[64 * half:64 * half + 64, off:off + TILE],
                        start=True, stop=True,
                    )
                    trash = trpool.tile([128, TILE], F32, tag="trash")
                    s_col = half * ct + t
                    nc.scalar.activation(
                        out=trash[:], in_=ps[:],
                        func=mybir.ActivationFunctionType.Relu,
                        bias=b1t[:, 0:1],
                        accum_out=S[:, s_col:s_col + 1],
                    )

            # transpose S into [tiles, hid] chunks and combine with amat
            st_chunks = []
            for k in range(nch):
                pst = pscpool.tile([128, 128], F32, tag="pst")
                nc.tensor.transpose(pst[:], S[:, 128 * k:128 * k + 128], ident[:])
                st = trpool.tile([128, 128], F32, tag=f"st{k}")
                nc.vector.tensor_copy(st[:], pst[:])
                st_chunks.append(st)
            out_sb = trpool.tile([128, NSEG], F32, tag="out_sb")
            for j in range(NSEG // 512):
                pss = pscpool.tile([128, 512], F32, tag="pss")
                for k in range(nch):
                    nc.tensor.matmul(
                        pss[:], lhsT=st_chunks[k][:],
                        rhs=amat[:, k, 512 * j:512 * j + 512],
                        start=(k == 0), stop=(k == nch - 1),
                    )
                nc.vector.tensor_copy(out_sb[:, 512 * j:512 * j + 512], pss[:])
            nc.sync.dma_start(ps_d[:], out_sb[:])

    nc.compile()
    return nc


def _build_phase2():
    nc = bacc.Bacc("TRN2", target_bir_lowering=False, debug=False, num_devices=1)
    gs_d = nc.dram_tensor("gsums", [128, NSEG], F32, kind="ExternalInput").ap()
    invc_d = nc.dram_tensor("invc", [128, NSEG], F32, kind="ExternalInput").ap()
    b2nz_d = nc.dram_tensor("b2nz", [128, NSEG], F32, kind="ExternalInput").ap()
    w2_d = nc.dram_tensor("w2", [128, 128], F32, kind="ExternalInput").ap()
    w3_d = nc.dram_tensor("w3", [128, 128], F32, kind="ExternalInput").ap()
    w4_d = nc.dram_tensor("w4", [128, ODIM], F32, kind="ExternalInput").ap()
    b3_d = nc.dram_tensor("b3", [128, 1], F32, kind="ExternalInput").ap()
    b4_d = nc.dram_tensor("b4", [ODIM, 1], F32, kind="ExternalInput").ap()
    out_d = nc.dram_tensor("out_t", [ODIM, NSEG], F32, kind="ExternalOutput").ap()

    with tile.TileContext(nc) as tc:
        with tc.tile_pool(name="sb", bufs=1) as pool, \
             tc.tile_pool(name="ps", bufs=2, space="PSUM") as psp:
            gs = pool.tile([128, NSEG], F32)
            nc.sync.dma_start(gs[:], gs_d[:])
            invc = pool.tile([128, NSEG], F32)
            nc.sync.dma_start(invc[:], invc_d[:])
            b2nz = pool.tile([128, NSEG], F32)
            nc.sync.dma_start(b2nz[:], b2nz_d[:])
            w2 = pool.tile([128, 128], F32)
            nc.sync.dma_start(w2[:], w2_d[:])
            w3 = pool.tile([128, 128], F32)
            nc.sync.dma_start(w3[:], w3_d[:])
            w4 = pool.tile([128, ODIM], F32)
            nc.sync.dma_start(w4[:], w4_d[:])
            b3 = pool.tile([128, 1], F32)
            nc.sync.dma_start(b3[:], b3_d[:])
            b4 = pool.tile([ODIM, 1], F32)
            nc.sync.dma_start(b4[:], b4_d[:])

            mean = pool.tile([128, NSEG], F32)
            nc.vector.tensor_tensor(out=mean[:], in0=gs[:], in1=invc[:],
                                    op=mybir.AluOpType.mult)
            hid = pool.tile([128, NSEG], F32)
            t3 = pool.tile([128, NSEG], F32)
            ot = pool.tile([ODIM, NSEG], F32)
            for j in range(NSEG // 512):
                sl = slice(512 * j, 512 * j + 512)
                p2 = psp.tile([128, 512], F32, tag="p")
                nc.tensor.matmul(p2[:], lhsT=w2[:], rhs=mean[:, sl],
                                 start=True, stop=True)
                nc.vector.tensor_tensor(out=hid[:, sl], in0=p2[:],
                                        in1=b2nz[:, sl], op=mybir.AluOpType.add)
            for j in range(NSEG // 512):
                sl = slice(512 * j, 512 * j + 512)
                p3 = psp.tile([128, 512], F32, tag="p")
                nc.tensor.matmul(p3[:], lhsT=w3[:], rhs=hid[:, sl],
                                 start=True, stop=True)
                nc.scalar.activation(out=t3[:, sl], in_=p3[:],
                                     func=mybir.ActivationFunctionType.Relu,
                                     bias=b3[:, 0:1])
            for j in range(NSEG // 512):
                sl = slice(512 * j, 512 * j + 512)
                p4f = psp.tile([128, 512], F32, tag="p")
                p4 = p4f[:ODIM, :]
                nc.tensor.matmul(p4, lhsT=w4[:], rhs=t3[:, sl],
                                 start=True, stop=True)
                nc.scalar.activation(out=ot[:, sl], in_=p4,
                                     func=mybir.ActivationFunctionType.Identity,
                                     bias=b4[:, 0:1])
            nc.sync.dma_start(out_d[:], ot[:])
    nc.compile()
    return nc


def run(inputs, ncores=NCORES, trace=False):
    x = np.asarray(inputs["x"], dtype=np.float32)
    xb = np.asarray(inputs["x_batch"])
    W1 = np.asarray(inputs["W1"], dtype=np.float32)
    b1 = np.asarray(inputs["b1"], dtype=np.float32)
    in_maps, meta = _host_prep(x, xb, W1, b1, ncores=ncores)

    nc1 = _build_phase1(meta)
    res1 = run_bass_kernel_spmd(nc1, in_maps, core_ids=list(range(ncores)),
                                trace=trace)
    gsums = np.zeros((128, NSEG), dtype=np.float64)
    for c in range(ncores):
        gsums += res1.results[c]["psums"].astype(np.float64)
    # remove the relu(b1) contribution of zero-pad columns
    gsums -= np.maximum(b1, 0.0)[:, None].astype(np.float64) * meta["padcount"][None, :]
    gsums = gsums.astype(np.float32)

    counts = meta["counts"]
    invc = (1.0 / np.maximum(counts, 1.0)).astype(np.float32)
    nz = (counts > 0).astype(np.float32)
    b2 = np.asarray(inputs["b2"], dtype=np.float32)
    p2_ins = [dict(
        gsums=gsums,
        invc=np.ascontiguousarray(np.broadcast_to(invc, (128, NSEG))),
        b2nz=np.ascontiguousarray(b2.reshape(128, 1) * nz[None, :]),
        w2=np.ascontiguousarray(inputs["W2"], dtype=np.float32),
        w3=np.ascontiguousarray(inputs["W3"], dtype=np.float32),
        w4=np.ascontiguousarray(inputs["W4"], dtype=np.float32),
        b3=np.ascontiguousarray(inputs["b3"], dtype=np.float32).reshape(128, 1),
        b4=np.ascontiguousarray(inputs["b4"], dtype=np.float32).reshape(ODIM, 1),
    )]
    nc2 = _build_phase2()
    res2 = run_bass_kernel_spmd(nc2, p2_ins, core_ids=[0], trace=trace)
    out = np.ascontiguousarray(res2.results[0]["out_t"].T).astype(np.float32)
    return out, res1, res2


def kernel(**inputs):
    inputs = {k: np.asarray(v) for k, v in inputs.items()}
    out, _, _ = run(inputs)
    return out


if __name__ == "__main__":
    rng = np.random.default_rng(0)
    N, D, HN, B = 8 * 2 * 2 * TILE, 64, 128, 64
    x = rng.standard_normal((N, D), dtype=np.float32)
    xb = np.sort(rng.integers(0, B, N).astype(np.int32))
    W1 = (rng.standard_normal((D, HN)) / 8).astype(np.float32)
    W2 = (rng.standard_normal((HN, HN)) / 11.3).astype(np.float32)
    W3 = (rng.standard_normal((HN, HN)) / 11.3).astype(np.float32)
    W4 = (rng.standard_normal((HN, ODIM)) / 11.3).astype(np.float32)
    b1 = rng.standard_normal(HN).astype(np.float32) * 0.1
    b2 = rng.standard_normal(HN).astype(np.float32) * 0.1
    b3 = rng.standard_normal(HN).astype(np.float32) * 0.1
    b4 = rng.standard_normal(ODIM).astype(np.float32) * 0.1
    ins = dict(x=x, x_batch=xb, W1=W1, b1=b1, W2=W2, b2=b2, W3=W3, b3=b3,
               W4=W4, b4=b4)
    out = kernel(**ins)

    h = np.maximum(x @ W1 + b1, 0) @ W2 + b2
    sums = np.zeros((1024, HN), dtype=np.float64)
    np.add.at(sums, xb, h.astype(np.float64))
    cnt = np.bincount(xb, minlength=1024).astype(np.float64)
    mean = sums / np.maximum(cnt, 1)[:, None]
    ref = (np.maximum(mean @ W3 + b3, 0) @ W4 + b4).astype(np.float32)
    num = np.linalg.norm(out - ref)
    den = np.linalg.norm(ref)
    print("Relative error:", num / den)



# revision 8
# speedup vs baseline: 1.1736x; 1.1736x over previous
"""DeepSet segment-reduce kernel for 8 Trainium2 NeuronCores.

Math (equivalent to the reference, using linearity of segment_sum):
    r      = relu(x @ W1 + b1)                      # per-node, on device
    sums_r = segment_sum(r)                         # [B, HID]
    mean_r = sums_r / max(counts, 1)                # counts via host bincount
    hid    = mean_r @ W2 + b2                       # tiny tail, on device
    out    = relu(hid @ W3 + b3) @ W4 + b4          # tiny tail, on device

Phase 1 (8 cores, data-parallel over nodes): each core gets a contiguous
shard of ~N/8 nodes laid out one node per SBUF column (64 features in
partitions 0..63, bf16).  The host pads every segment's node run to a
multiple of 2048 columns, so each 2048-column superblock belongs to exactly
one segment.  Per superblock: 4 bf16 matmuls (K=64, N=512, single resident
W1) fill a 4-bank PSUM tile, then ONE fused relu+bias+sum instruction
reduces it to a [128,1] segment-partial.  Superblocks alternate between the
Scalar engine (ACT Relu with accum_out, ~2181 ns) and the Vector engine
(tensor_scalar add-bias/max0 with accum_out, ~2395 ns) so both elementwise
engines run flat out in parallel; the PE (4x216 ns per superblock) and DMA
(~1 MB/4 superblocks) pipeline underneath.

The host routes superblock partials to segments (trivial bookkeeping over
[128, n_sb] outputs), removes the pad columns' relu(b1) contribution,
applies the mean, and a second tiny NEFF runs the rho MLP in bf16.
"""

import os
import sys

for _p in ("/opt/trn_rl_repo",):
    if os.path.isdir(_p) and _p not in sys.path:
        sys.path.append(_p)

import numpy as np
import ml_dtypes

import concourse.bass as bass
import concourse.tile as tile
from concourse import bacc, mybir
from concourse.bass_utils import run_bass_kernel_spmd

F32 = mybir.dt.float32
BF16 = mybir.dt.bfloat16

NCORES = 8
TILE = 512
SB = 2048            # superblock columns (4 PSUM banks)
SB_PER_CHUNK = 4     # superblocks per DMA chunk
NSEG = 1024
ODIM = 16

ACT_NS = 2181        # measured per-superblock reduce cost on Scalar
DVE_NS = 2395        # measured per-superblock reduce cost on Vector


def _assign_engines(n_sb):
    """Greedy A/D assignment balancing measured per-superblock cost."""
    ta = td = 0
    out = []
    for _ in range(n_sb):
        if ta + ACT_NS <= td + DVE_NS:
            out.append("A")
            ta += ACT_NS
        else:
            out.append("D")
            td += DVE_NS
    return out


def _host_prep(x, x_batch, ncores=NCORES):
    N = x.shape[0]
    assert N % ncores == 0
    ch = N // ncores
    xb = np.asarray(x_batch)

    counts = np.bincount(xb, minlength=NSEG).astype(np.float64)

    plans = []
    n_sb_max = 0
    for c in range(ncores):
        lo = c * ch
        ids = xb[lo:lo + ch]
        uniq, starts = np.unique(ids, return_index=True)
        ends = np.append(starts[1:], ch)
        seg_of_sb = []
        src_parts = []
        for k in range(len(uniq)):
            L = int(ends[k] - starts[k])
            T = -(-L // SB)
            arr = np.full(T * SB, -1, dtype=np.int64)
            arr[:L] = lo + starts[k] + np.arange(L)
            src_parts.append(arr)
            seg_of_sb += [int(uniq[k])] * T
        src = np.concatenate(src_parts) if src_parts else np.empty(0, np.int64)
        plans.append((src, seg_of_sb))
        n_sb_max = max(n_sb_max, len(seg_of_sb))

    cols = n_sb_max * SB
    padcount = np.zeros(NSEG, dtype=np.float64)
    in_maps = []
    xts = []
    for c in range(ncores):
        src, seg_of_sb = plans[c]
        if len(src) < cols:
            src = np.concatenate([src, np.full(cols - len(src), -1, np.int64)])
        mask = src >= 0
        gath = np.zeros((cols, 64), dtype=np.float32)
        gath[mask] = x[src[mask]]
        xt = np.ascontiguousarray(gath.T).astype(ml_dtypes.bfloat16)
        xts.append(xt)
        # pad columns inside real (routed) superblocks contribute relu(b1)
        if seg_of_sb:
            seg_arr = np.array(seg_of_sb, dtype=np.int64)
            real = mask[:len(seg_arr) * SB].reshape(-1, SB).sum(axis=1)
            np.add.at(padcount, seg_arr, SB - real)
    meta = dict(n_sb=n_sb_max, cols=cols, counts=counts, padcount=padcount,
                seg_of_sb=[p[1] for p in plans], ncores=ncores)
    return xts, meta


def _build_phase1(n_sb, cols, assign, ncores=NCORES):
    nc = bacc.Bacc("TRN2", target_bir_lowering=False, debug=False,
                   num_devices=ncores)
    xt_d = nc.dram_tensor("xt", [64, cols], BF16, kind="ExternalInput").ap()
    w1_d = nc.dram_tensor("w1t", [64, 128], BF16, kind="ExternalInput").ap()
    b1_d = nc.dram_tensor("b1", [128, 1], F32, kind="ExternalInput").ap()
    nb1_d = nc.dram_tensor("nb1", [128, 1], F32, kind="ExternalInput").ap()
    sa_d = nc.dram_tensor("s_act", [128, n_sb], F32, kind="ExternalOutput").ap()
    sd_d = nc.dram_tensor("s_dve", [128, n_sb], F32, kind="ExternalOutput").ap()

    CH = SB_PER_CHUNK * SB

    with tile.TileContext(nc) as tc:
        with tc.tile_pool(name="const", bufs=1) as cpool, \
             tc.tile_pool(name="xin", bufs=3) as xpool, \
             tc.tile_pool(name="tr", bufs=1) as trpool, \
             tc.tile_pool(name="ps", bufs=1, space="PSUM") as pspool:

            w1t = cpool.tile([64, 128], BF16)
            nc.sync.dma_start(w1t[:], w1_d[:])
            b1t = cpool.tile([128, 1], F32)
            nc.sync.dma_start(b1t[:], b1_d[:])
            nb1t = cpool.tile([128, 1], F32)
            nc.sync.dma_start(nb1t[:], nb1_d[:])
            S_a = cpool.tile([128, n_sb], F32)
            nc.vector.memset(S_a[:], 0.0)
            S_d = cpool.tile([128, n_sb], F32)
            nc.vector.memset(S_d[:], 0.0)

            xtile = None
            for sb in range(n_sb):
                if sb % SB_PER_CHUNK == 0:
                    w = min(CH, cols - sb * SB)
                    xtile = xpool.tile([64, CH], BF16, tag="x")
                    nc.sync.dma_start(xtile[:, :w], xt_d[:, sb * SB:sb * SB + w])
                base = (sb % SB_PER_CHUNK) * SB
                ps = pspool.tile([128, SB], F32,
                                 tag=("psa" if sb % 2 == 0 else "psb"))
                for t in range(SB // TILE):
                    off = base + t * TILE
                    nc.tensor.matmul(
                        ps[:, t * TILE:t * TILE + TILE],
                        lhsT=w1t[:],
                        rhs=xtile[:, off:off + TILE],
                        start=True, stop=True,
                    )
                if assign[sb] == "A":
                    trash = trpool.tile([128, SB], BF16, tag="ta")
                    nc.scalar.activation(
                        out=trash[:], in_=ps[:],
                        func=mybir.ActivationFunctionType.Relu,
                        bias=b1t[:, 0:1],
                        accum_out=S_a[:, sb:sb + 1],
                    )
                else:
                    trash = trpool.tile([128, SB], BF16, tag="td")
                    # accum_out = add-reduce of max(psum, -b1)
                    #           = sum(relu(psum + b1)) - SB*b1  (host adds it back)
                    nc.vector.tensor_scalar(
                        out=trash[:], in0=ps[:],
                        scalar1=nb1t[:, 0:1], scalar2=0.0,
                        op0=mybir.AluOpType.max, op1=mybir.AluOpType.add,
                        accum_out=S_d[:, sb:sb + 1],
                    )

            nc.sync.dma_start(sa_d[:], S_a[:])
            nc.sync.dma_start(sd_d[:], S_d[:])

    nc.compile()
    return nc


def _build_phase2():
    nc = bacc.Bacc("TRN2", target_bir_lowering=False, debug=False, num_devices=1)
    mean_d = nc.dram_tensor("mean", [128, NSEG], BF16, kind="ExternalInput").ap()
    w2_d = nc.dram_tensor("w2", [128, 128], BF16, kind="ExternalInput").ap()
    w3_d = nc.dram_tensor("w3", [128, 128], BF16, kind="ExternalInput").ap()
    w4_d = nc.dram_tensor("w4", [128, ODIM], BF16, kind="ExternalInput").ap()
    b2_d = nc.dram_tensor("b2", [128, 1], F32, kind="ExternalInput").ap()
    b3_d = nc.dram_tensor("b3", [128, 1], F32, kind="ExternalInput").ap()
    b4_d = nc.dram_tensor("b4", [ODIM, 1], F32, kind="ExternalInput").ap()
    out_d = nc.dram_tensor("out_t", [ODIM, NSEG], F32, kind="ExternalOutput").ap()

    with tile.TileContext(nc) as tc:
        with tc.tile_pool(name="sb", bufs=1) as pool, \
             tc.tile_pool(name="ps", bufs=2, space="PSUM") as psp:
            mean = pool.tile([128, NSEG], BF16)
            nc.sync.dma_start(mean[:], mean_d[:])
            w2 = pool.tile([128, 128], BF16)
            nc.sync.dma_start(w2[:], w2_d[:])
            w3 = pool.tile([128, 128], BF16)
            nc.sync.dma_start(w3[:], w3_d[:])
            w4 = pool.tile([128, ODIM], BF16)
            nc.sync.dma_start(w4[:], w4_d[:])
            b2 = pool.tile([128, 1], F32)
            nc.sync.dma_start(b2[:], b2_d[:])
            b3 = pool.tile([128, 1], F32)
            nc.sync.dma_start(b3[:], b3_d[:])
            b4 = pool.tile([ODIM, 1], F32)
            nc.sync.dma_start(b4[:], b4_d[:])

            hid = pool.tile([128, NSEG], BF16)
            t3 = pool.tile([128, NSEG], BF16)
            ot = pool.tile([ODIM, NSEG], F32)
            for j in range(NSEG // 512):
                sl = slice(512 * j, 512 * j + 512)
                p2 = psp.tile([128, 512], F32, tag="p")
                nc.tensor.matmul(p2[:], lhsT=w2[:], rhs=mean[:, sl],
                                 start=True, stop=True)
                nc.scalar.activation(out=hid[:, sl], in_=p2[:],
                                     func=mybir.ActivationFunctionType.Identity,
                                     bias=b2[:, 0:1])
            for j in range(NSEG // 512):
                sl = slice(512 * j, 512 * j + 512)
                p3 = psp.tile([128, 512], F32, tag="p")
                nc.tensor.matmul(p3[:], lhsT=w3[:], rhs=hid[:, sl],
                                 start=True, stop=True)
                nc.scalar.activation(out=t3[:, sl], in_=p3[:],
                                     func=mybir.ActivationFunctionType.Relu,
                                     bias=b3[:, 0:1])
            for j in range(NSEG // 512):
                sl = slice(512 * j, 512 * j + 512)
                p4f = psp.tile([128, 512], F32, tag="p")
                p4 = p4f[:ODIM, :]
                nc.tensor.matmul(p4, lhsT=w4[:], rhs=t3[:, sl],
                                 start=True, stop=True)
                nc.scalar.activation(out=ot[:, sl], in_=p4,
                                     func=mybir.ActivationFunctionType.Identity,
                                     bias=b4[:, 0:1])
            nc.sync.dma_start(out_d[:], ot[:])
    nc.compile()
    return nc


def run(inputs, ncores=NCORES, trace=False):
    x = np.asarray(inputs["x"], dtype=np.float32)
    xb = np.asarray(inputs["x_batch"])
    W1 = np.asarray(inputs["W1"], dtype=np.float32)
    b1 = np.asarray(inputs["b1"], dtype=np.float32)

    xts, meta = _host_prep(x, xb, ncores=ncores)
    n_sb, cols = meta["n_sb"], meta["cols"]
    assign = _assign_engines(n_sb)

    w1t = np.ascontiguousarray(W1).astype(ml_dtypes.bfloat16)       # [64,128]
    b1c = np.ascontiguousarray(b1, np.float32).reshape(128, 1)
    nb1c = np.ascontiguousarray(-b1, np.float32).reshape(128, 1)
    in_maps = [dict(xt=xts[c], w1t=w1t, b1=b1c, nb1=nb1c) for c in range(ncores)]

    nc1 = _build_phase1(n_sb, cols, assign, ncores=ncores)
    res1 = run_bass_kernel_spmd(nc1, in_maps, core_ids=list(range(ncores)),
                                trace=trace)

    # host: route superblock partials to segments, 8-core combine.
    # DVE superblocks computed sum(max(psum,-b1)) = sum(relu(psum+b1)) - SB*b1.
    is_dve = np.array([a == "D" for a in assign])
    b1f = b1.astype(np.float64)
    gsums = np.zeros((NSEG, 128), dtype=np.float64)
    for c in range(ncores):
        S = (res1.results[c]["s_act"].astype(np.float64)
             + res1.results[c]["s_dve"].astype(np.float64))   # [128, n_sb]
        seg = np.array(meta["seg_of_sb"][c], dtype=np.int64)
        if len(seg):
            St = S.T[:len(seg)].copy()
            St[is_dve[:len(seg)]] += SB * b1f[None, :]
            np.add.at(gsums, seg, St)
    # remove the relu(b1) contribution of zero-pad columns
    gsums -= np.maximum(b1, 0.0)[None, :].astype(np.float64) * meta["padcount"][:, None]

    counts = meta["counts"]
    mean = gsums / np.maximum(counts, 1.0)[:, None]                  # [NSEG,128]

    p2_ins = [dict(
        mean=np.ascontiguousarray(mean.T.astype(ml_dtypes.bfloat16)),
        w2=np.ascontiguousarray(inputs["W2"], np.float32).astype(ml_dtypes.bfloat16),
        w3=np.ascontiguousarray(inputs["W3"], np.float32).astype(ml_dtypes.bfloat16),
        w4=np.ascontiguousarray(inputs["W4"], np.float32).astype(ml_dtypes.bfloat16),
        b2=np.ascontiguousarray(inputs["b2"], np.float32).reshape(128, 1),
        b3=np.ascontiguousarray(inputs["b3"], np.float32).reshape(128, 1),
        b4=np.ascontiguousarray(inputs["b4"], np.float32).reshape(ODIM, 1),
    )]
    nc2 = _build_phase2()
    res2 = run_bass_kernel_spmd(nc2, p2_ins, core_ids=[0], trace=trace)
    out = np.ascontiguousarray(res2.results[0]["out_t"].T).astype(np.float32)

    # segments with no nodes: reference's hid is 0 (not b2), so
    # out = relu(b3) @ W4 + b4 exactly
    empty = counts == 0
    if empty.any():
        row = (np.maximum(np.asarray(inputs["b3"], np.float64), 0.0)
               @ np.asarray(inputs["W4"], np.float64)
               + np.asarray(inputs["b4"], np.float64))
        out[empty] = row.astype(np.float32)
    return out, res1, res2


def kernel(**inputs):
    inputs = {k: np.asarray(v) for k, v in inputs.items()}
    out, _, _ = run(inputs)
    return out


if __name__ == "__main__":
    rng = np.random.default_rng(0)
    N, D, HN, B = 8 * 16 * SB, 64, 128, 64
    x = rng.standard_normal((N, D), dtype=np.float32)
    xb = np.sort(rng.integers(0, B, N).astype(np.int32))
    W1 = (rng.standard_normal((D, HN)) / 8).astype(np.float32)
    W2 = (rng.standard_normal((HN, HN)) / 11.3).astype(np.float32)
    W3 = (rng.standard_normal((HN, HN)) / 11.3).astype(np.float32)
    W4 = (rng.standard_normal((HN, ODIM)) / 11.3).astype(np.float32)
    b1 = rng.standard_normal(HN).astype(np.float32) * 0.1
    b2 = rng.standard_normal(HN).astype(np.float32) * 0.1
    b3 = rng.standard_normal(HN).astype(np.float32) * 0.1
    b4 = rng.standard_normal(ODIM).astype(np.float32) * 0.1
    ins = dict(x=x, x_batch=xb, W1=W1, b1=b1, W2=W2, b2=b2, W3=W3, b3=b3,
               W4=W4, b4=b4)
    out = kernel(**ins)

    h = np.maximum(x @ W1 + b1, 0) @ W2 + b2
    sums = np.zeros((1024, HN), dtype=np.float64)
    np.add.at(sums, xb, h.astype(np.float64))
    cnt = np.bincount(xb, minlength=1024).astype(np.float64)
    mean = sums / np.maximum(cnt, 1)[:, None]
    ref = (np.maximum(mean @ W3 + b3, 0) @ W4 + b4).astype(np.float32)
    num = np.linalg.norm(out - ref)
    den = np.linalg.norm(ref)
    print("Relative error:", num / den)


# revision 10
# speedup vs baseline: 1.7351x; 1.4784x over previous
"""DeepSet segment-reduce kernel for 8 Trainium2 NeuronCores.

Math (equivalent to the reference, using linearity of segment_sum):
    r      = relu(x @ W1 + b1)                      # per-node, on device
    sums_r = segment_sum(r)                         # [B, HID]
    mean_r = sums_r / max(counts, 1)                # counts via host bincount
    hid    = mean_r @ W2 + b2                       # tiny tail, on device
    out    = relu(hid @ W3 + b3) @ W4 + b4          # tiny tail, on device

Phase 1 (8 cores, data-parallel over nodes): each core gets a contiguous
shard of ~N/8 nodes laid out one node per SBUF column (64 features in
partitions 0..63, bf16).  The host pads every segment's node run to a
multiple of 2048 columns, so each 2048-column superblock belongs to exactly
one segment.  Per superblock: 4 bf16 matmuls (K=64, N=512, single resident
W1) fill a 4-bank PSUM tile, then ONE fused relu+bias+sum instruction
reduces it to a [128,1] segment-partial.  Superblocks alternate between the
Scalar engine (ACT Relu with accum_out, ~2181 ns) and the Vector engine
(tensor_scalar add-bias/max0 with accum_out, ~2395 ns) so both elementwise
engines run flat out in parallel; the PE (4x216 ns per superblock) and DMA
(~1 MB/4 superblocks) pipeline underneath.

The host routes superblock partials to segments (trivial bookkeeping over
[128, n_sb] outputs), removes the pad columns' relu(b1) contribution,
applies the mean, and a second tiny NEFF runs the rho MLP in bf16.
"""

import os
import sys

for _p in ("/opt/trn_rl_repo",):
    if os.path.isdir(_p) and _p not in sys.path:
        sys.path.append(_p)

import numpy as np
import ml_dtypes

import concourse.bass as bass
import concourse.tile as tile
from concourse import bacc, mybir
from concourse.bass_utils import run_bass_kernel_spmd

F32 = mybir.dt.float32
BF16 = mybir.dt.bfloat16

NCORES = 8
TILE = 512
SB = 1024            # superblock columns (2 PSUM banks; 4 tiles in flight)
SB_PER_CHUNK = 8     # superblocks per DMA chunk
NSEG = 1024
ODIM = 16


def _assign_engines(n_sb):
    """Strict Scalar/Vector alternation; each engine owns 2 of the 4 PSUM
    tiles so reduce(sb) overlaps the matmul refill of its engine's other
    tile."""
    return ["A" if i % 2 == 0 else "D" for i in range(n_sb)]


def _host_prep(x, x_batch, ncores=NCORES):
    N = x.shape[0]
    assert N % ncores == 0
    ch = N // ncores
    xb = np.asarray(x_batch)

    counts = np.bincount(xb, minlength=NSEG).astype(np.float64)

    plans = []
    n_sb_max = 0
    for c in range(ncores):
        lo = c * ch
        ids = xb[lo:lo + ch]
        uniq, starts = np.unique(ids, return_index=True)
        ends = np.append(starts[1:], ch)
        seg_of_sb = []
        src_parts = []
        for k in range(len(uniq)):
            L = int(ends[k] - starts[k])
            T = -(-L // SB)
            arr = np.full(T * SB, -1, dtype=np.int64)
            arr[:L] = lo + starts[k] + np.arange(L)
            src_parts.append(arr)
            seg_of_sb += [int(uniq[k])] * T
        src = np.concatenate(src_parts) if src_parts else np.empty(0, np.int64)
        plans.append((src, seg_of_sb))
        n_sb_max = max(n_sb_max, len(seg_of_sb))

    cols = n_sb_max * SB
    padcount = np.zeros(NSEG, dtype=np.float64)
    in_maps = []
    xts = []
    for c in range(ncores):
        src, seg_of_sb = plans[c]
        if len(src) < cols:
            src = np.concatenate([src, np.full(cols - len(src), -1, np.int64)])
        mask = src >= 0
        gath = np.zeros((cols, 64), dtype=np.float32)
        gath[mask] = x[src[mask]]
        xt = np.ascontiguousarray(gath.T).astype(ml_dtypes.bfloat16)
        xts.append(xt)
        # pad columns inside real (routed) superblocks contribute relu(b1)
        if seg_of_sb:
            seg_arr = np.array(seg_of_sb, dtype=np.int64)
            real = mask[:len(seg_arr) * SB].reshape(-1, SB).sum(axis=1)
            np.add.at(padcount, seg_arr, SB - real)
    meta = dict(n_sb=n_sb_max, cols=cols, counts=counts, padcount=padcount,
                seg_of_sb=[p[1] for p in plans], ncores=ncores)
    return xts, meta


def _build_phase1(n_sb, cols, assign, ncores=NCORES):
    nc = bacc.Bacc("TRN2", target_bir_lowering=False, debug=False,
                   num_devices=ncores)
    xt_d = nc.dram_tensor("xt", [64, cols], BF16, kind="ExternalInput").ap()
    w1_d = nc.dram_tensor("w1t", [64, 128], BF16, kind="ExternalInput").ap()
    b1_d = nc.dram_tensor("b1", [128, 1], F32, kind="ExternalInput").ap()
    nb1_d = nc.dram_tensor("nb1", [128, 1], F32, kind="ExternalInput").ap()
    sa_d = nc.dram_tensor("s_act", [128, n_sb], F32, kind="ExternalOutput").ap()
    sd_d = nc.dram_tensor("s_dve", [128, n_sb], F32, kind="ExternalOutput").ap()

    CH = SB_PER_CHUNK * SB

    with tile.TileContext(nc) as tc:
        with tc.tile_pool(name="const", bufs=1) as cpool, \
             tc.tile_pool(name="xin", bufs=3) as xpool, \
             tc.tile_pool(name="tr", bufs=1) as trpool, \
             tc.tile_pool(name="ps", bufs=1, space="PSUM") as pspool:

            w1t = cpool.tile([64, 128], BF16)
            nc.sync.dma_start(w1t[:], w1_d[:])
            b1t = cpool.tile([128, 1], F32)
            nc.sync.dma_start(b1t[:], b1_d[:])
            nb1t = cpool.tile([128, 1], F32)
            nc.sync.dma_start(nb1t[:], nb1_d[:])
            S_a = cpool.tile([128, n_sb], F32)
            nc.vector.memset(S_a[:], 0.0)
            S_d = cpool.tile([128, n_sb], F32)
            nc.vector.memset(S_d[:], 0.0)

            xtile = None
            for sb in range(n_sb):
                if sb % SB_PER_CHUNK == 0:
                    w = min(CH, cols - sb * SB)
                    xtile = xpool.tile([64, CH], BF16, tag="x")
                    nc.sync.dma_start(xtile[:, :w], xt_d[:, sb * SB:sb * SB + w])
                base = (sb % SB_PER_CHUNK) * SB
                ps = pspool.tile([128, SB], F32, tag=f"ps{sb % 4}")
                for t in range(SB // TILE):
                    off = base + t * TILE
                    nc.tensor.matmul(
                        ps[:, t * TILE:t * TILE + TILE],
                        lhsT=w1t[:],
                        rhs=xtile[:, off:off + TILE],
                        start=True, stop=True,
                    )
                if assign[sb] == "A":
                    trash = trpool.tile([128, SB], BF16, tag="ta")
                    nc.scalar.activation(
                        out=trash[:], in_=ps[:],
                        func=mybir.ActivationFunctionType.Relu,
                        bias=b1t[:, 0:1],
                        accum_out=S_a[:, sb:sb + 1],
                    )
                else:
                    trash = trpool.tile([128, SB], BF16, tag="td")
                    # accum_out = add-reduce of max(psum, -b1)
                    #           = sum(relu(psum + b1)) - SB*b1  (host adds it back)
                    nc.vector.tensor_scalar(
                        out=trash[:], in0=ps[:],
                        scalar1=nb1t[:, 0:1], scalar2=0.0,
                        op0=mybir.AluOpType.max, op1=mybir.AluOpType.add,
                        accum_out=S_d[:, sb:sb + 1],
                    )

            nc.sync.dma_start(sa_d[:], S_a[:])
            nc.sync.dma_start(sd_d[:], S_d[:])

    nc.compile()
    return nc


def _build_phase2():
    nc = bacc.Bacc("TRN2", target_bir_lowering=False, debug=False, num_devices=1)
    mean_d = nc.dram_tensor("mean", [128, NSEG], BF16, kind="ExternalInput").ap()
    w2_d = nc.dram_tensor("w2", [128, 128], BF16, kind="ExternalInput").ap()
    w3_d = nc.dram_tensor("w3", [128, 128], BF16, kind="ExternalInput").ap()
    w4_d = nc.dram_tensor("w4", [128, ODIM], BF16, kind="ExternalInput").ap()
    b2_d = nc.dram_tensor("b2", [128, 1], F32, kind="ExternalInput").ap()
    b3_d = nc.dram_tensor("b3", [128, 1], F32, kind="ExternalInput").ap()
    b4_d = nc.dram_tensor("b4", [ODIM, 1], F32, kind="ExternalInput").ap()
    out_d = nc.dram_tensor("out_t", [ODIM, NSEG], F32, kind="ExternalOutput").ap()

    with tile.TileContext(nc) as tc:
        with tc.tile_pool(name="sb", bufs=1) as pool, \
             tc.tile_pool(name="ps", bufs=2, space="PSUM") as psp:
            mean = pool.tile([128, NSEG], BF16)
            nc.sync.dma_start(mean[:], mean_d[:])
            w2 = pool.tile([128, 128], BF16)
            nc.sync.dma_start(w2[:], w2_d[:])
            w3 = pool.tile([128, 128], BF16)
            nc.sync.dma_start(w3[:], w3_d[:])
            w4 = pool.tile([128, ODIM], BF16)
            nc.sync.dma_start(w4[:], w4_d[:])
            b2 = pool.tile([128, 1], F32)
            nc.sync.dma_start(b2[:], b2_d[:])
            b3 = pool.tile([128, 1], F32)
            nc.sync.dma_start(b3[:], b3_d[:])
            b4 = pool.tile([ODIM, 1], F32)
            nc.sync.dma_start(b4[:], b4_d[:])

            hid = pool.tile([128, NSEG], BF16)
            t3 = pool.tile([128, NSEG], BF16)
            ot = pool.tile([ODIM, NSEG], F32)
            for j in range(NSEG // 512):
                sl = slice(512 * j, 512 * j + 512)
                p2 = psp.tile([128, 512], F32, tag="p")
                nc.tensor.matmul(p2[:], lhsT=w2[:], rhs=mean[:, sl],
                                 start=True, stop=True)
                nc.scalar.activation(out=hid[:, sl], in_=p2[:],
                                     func=mybir.ActivationFunctionType.Identity,
                                     bias=b2[:, 0:1])
            for j in range(NSEG // 512):
                sl = slice(512 * j, 512 * j + 512)
                p3 = psp.tile([128, 512], F32, tag="p")
                nc.tensor.matmul(p3[:], lhsT=w3[:], rhs=hid[:, sl],
                                 start=True, stop=True)
                nc.scalar.activation(out=t3[:, sl], in_=p3[:],
                                     func=mybir.ActivationFunctionType.Relu,
                                     bias=b3[:, 0:1])
            for j in range(NSEG // 512):
                sl = slice(512 * j, 512 * j + 512)
                p4f = psp.tile([128, 512], F32, tag="p")
                p4 = p4f[:ODIM, :]
                nc.tensor.matmul(p4, lhsT=w4[:], rhs=t3[:, sl],
                                 start=True, stop=True)
                nc.scalar.activation(out=ot[:, sl], in_=p4,
                                     func=mybir.ActivationFunctionType.Identity,
                                     bias=b4[:, 0:1])
            nc.sync.dma_start(out_d[:], ot[:])
    nc.compile()
    return nc


def run(inputs, ncores=NCORES, trace=False):
    x = np.asarray(inputs["x"], dtype=np.float32)
    xb = np.asarray(inputs["x_batch"])
    W1 = np.asarray(inputs["W1"], dtype=np.float32)
    b1 = np.asarray(inputs["b1"], dtype=np.float32)

    xts, meta = _host_prep(x, xb, ncores=ncores)
    n_sb, cols = meta["n_sb"], meta["cols"]
    assign = _assign_engines(n_sb)

    w1t = np.ascontiguousarray(W1).astype(ml_dtypes.bfloat16)       # [64,128]
    b1c = np.ascontiguousarray(b1, np.float32).reshape(128, 1)
    nb1c = np.ascontiguousarray(-b1, np.float32).reshape(128, 1)
    in_maps = [dict(xt=xts[c], w1t=w1t, b1=b1c, nb1=nb1c) for c in range(ncores)]

    nc1 = _build_phase1(n_sb, cols, assign, ncores=ncores)
    res1 = run_bass_kernel_spmd(nc1, in_maps, core_ids=list(range(ncores)),
                                trace=trace)

    # host: route superblock partials to segments, 8-core combine.
    # DVE superblocks computed sum(max(psum,-b1)) = sum(relu(psum+b1)) - SB*b1.
    is_dve = np.array([a == "D" for a in assign])
    b1f = b1.astype(np.float64)
    gsums = np.zeros((NSEG, 128), dtype=np.float64)
    for c in range(ncores):
        S = (res1.results[c]["s_act"].astype(np.float64)
             + res1.results[c]["s_dve"].astype(np.float64))   # [128, n_sb]
        seg = np.array(meta["seg_of_sb"][c], dtype=np.int64)
        if len(seg):
            St = S.T[:len(seg)].copy()
            St[is_dve[:len(seg)]] += SB * b1f[None, :]
            np.add.at(gsums, seg, St)
    # remove the relu(b1) contribution of zero-pad columns
    gsums -= np.maximum(b1, 0.0)[None, :].astype(np.float64) * meta["padcount"][:, None]

    counts = meta["counts"]
    mean = gsums / np.maximum(counts, 1.0)[:, None]                  # [NSEG,128]

    p2_ins = [dict(
        mean=np.ascontiguousarray(mean.T.astype(ml_dtypes.bfloat16)),
        w2=np.ascontiguousarray(inputs["W2"], np.float32).astype(ml_dtypes.bfloat16),
        w3=np.ascontiguousarray(inputs["W3"], np.float32).astype(ml_dtypes.bfloat16),
        w4=np.ascontiguousarray(inputs["W4"], np.float32).astype(ml_dtypes.bfloat16),
        b2=np.ascontiguousarray(inputs["b2"], np.float32).reshape(128, 1),
        b3=np.ascontiguousarray(inputs["b3"], np.float32).reshape(128, 1),
        b4=np.ascontiguousarray(inputs["b4"], np.float32).reshape(ODIM, 1),
    )]
    nc2 = _build_phase2()
    res2 = run_bass_kernel_spmd(nc2, p2_ins, core_ids=[0], trace=trace)
    out = np.ascontiguousarray(res2.results[0]["out_t"].T).astype(np.float32)

    # segments with no nodes: reference's hid is 0 (not b2), so
    # out = relu(b3) @ W4 + b4 exactly
    empty = counts == 0
    if empty.any():
        row = (np.maximum(np.asarray(inputs["b3"], np.float64), 0.0)
               @ np.asarray(inputs["W4"], np.float64)
               + np.asarray(inputs["b4"], np.float64))
        out[empty] = row.astype(np.float32)
    return out, res1, res2


def kernel(**inputs):
    inputs = {k: np.asarray(v) for k, v in inputs.items()}
    out, _, _ = run(inputs)
    return out


if __name__ == "__main__":
    rng = np.random.default_rng(0)
    N, D, HN, B = 8 * 16 * SB, 64, 128, 64
    x = rng.standard_normal((N, D), dtype=np.float32)
    xb = np.sort(rng.integers(0, B, N).astype(np.int32))
    W1 = (rng.standard_normal((D, HN)) / 8).astype(np.float32)
    W2 = (rng.standard_normal((HN, HN)) / 11.3).astype(np.float32)
    W3 = (rng.standard_normal((HN, HN)) / 11.3).astype(np.float32)
    W4 = (rng.standard_normal((HN, ODIM)) / 11.3).astype(np.float32)
    b1 = rng.standard_normal(HN).astype(np.float32) * 0.1
    b2 = rng.standard_normal(HN).astype(np.float32) * 0.1
    b3 = rng.standard_normal(HN).astype(np.float32) * 0.1
    b4 = rng.standard_normal(ODIM).astype(np.float32) * 0.1
    ins = dict(x=x, x_batch=xb, W1=W1, b1=b1, W2=W2, b2=b2, W3=W3, b3=b3,
               W4=W4, b4=b4)
    out = kernel(**ins)

    h = np.maximum(x @ W1 + b1, 0) @ W2 + b2
    sums = np.zeros((1024, HN), dtype=np.float64)
    np.add.at(sums, xb, h.astype(np.float64))
    cnt = np.bincount(xb, minlength=1024).astype(np.float64)
    mean = sums / np.maximum(cnt, 1)[:, None]
    ref = (np.maximum(mean @ W3 + b3, 0) @ W4 + b4).astype(np.float32)
    num = np.linalg.norm(out - ref)
    den = np.linalg.norm(ref)
    print("Relative error:", num / den)


# revision 11
# speedup vs baseline: 1.9238x; 1.1088x over previous
"""DeepSet segment-reduce kernel for 8 Trainium2 NeuronCores.

Math (equivalent to the reference, using linearity of segment_sum):
    r      = relu(x @ W1 + b1)                      # per-node, on device
    sums_r = segment_sum(r)                         # [B, HID]
    mean_r = sums_r / max(counts, 1)                # counts via host bincount
    hid    = mean_r @ W2 + b2                       # tiny tail, on device
    out    = relu(hid @ W3 + b3) @ W4 + b4          # tiny tail, on device

Phase 1 (8 cores, data-parallel over nodes): each core's ~N/8 nodes are
split into two contiguous halves packed on SBUF partition halves (features
of half A in partitions 0..63, half B in 64..127, bf16), so DMA runs at
full 128-partition width.  Each half's segment runs are zero-padded to
multiples of 1024 columns, so every 1024-column superblock belongs to one
segment.  Matmuls use K=128 with zero-padded weights wzA=[[W1],[0]],
wzB=[[0],[W1]] — full-array matmuls keep the PE's HAM activity monitor
un-throttled at 2.4 GHz (K=64 matmuls leave it stuck at 1.2 GHz), and both
weight tiles live at PE tile position (0,0) (bf16 LDWEIGHTS at row tile 64
is broken in hardware).  Per superblock and half: 2 matmuls fill a 2-bank
PSUM tile, then ONE fused relu+bias+sum produces the [128,1] partial:
half A on the Scalar engine (ACT Relu, bias, accum_out), half B on the
Vector engine (tensor_scalar max(-b1)/add-reduce; the sum is off by
SB*b1, restored on the host).  With 4 PSUM tiles in flight both reduce
engines run continuously while the PE refills the other tiles.

The host routes superblock partials to segments, removes the pad columns'
relu(b1) contribution, applies the mean, and a second tiny NEFF runs the
rho MLP in bf16.  Segments with zero nodes are fixed up on the host
(reference gives relu(b3) @ W4 + b4 there).
"""

import os
import sys

for _p in ("/opt/trn_rl_repo",):
    if os.path.isdir(_p) and _p not in sys.path:
        sys.path.append(_p)

import numpy as np
import ml_dtypes

import concourse.bass as bass
import concourse.tile as tile
from concourse import bacc, mybir
from concourse.bass_utils import run_bass_kernel_spmd

F32 = mybir.dt.float32
BF16 = mybir.dt.bfloat16

NCORES = 8
TILE = 512
SB = 1024            # superblock columns (2 PSUM banks; 4 tiles in flight)
SB_PER_CHUNK = 8     # superblocks per DMA chunk
NSEG = 1024
ODIM = 16


def _pad_runs(ids, lo, ch):
    """Segment runs of a sorted id slice, padded to SB multiples.
    Returns (src indices with -1 pads, seg id per superblock)."""
    uniq, starts = np.unique(ids, return_index=True)
    ends = np.append(starts[1:], ch)
    seg_of_sb = []
    src_parts = []
    for k in range(len(uniq)):
        L = int(ends[k] - starts[k])
        T = -(-L // SB)
        arr = np.full(T * SB, -1, dtype=np.int64)
        arr[:L] = lo + starts[k] + np.arange(L)
        src_parts.append(arr)
        seg_of_sb += [int(uniq[k])] * T
    src = np.concatenate(src_parts) if src_parts else np.empty(0, np.int64)
    return src, seg_of_sb


def _host_prep(x, x_batch, ncores=NCORES):
    N = x.shape[0]
    assert N % (2 * ncores) == 0
    ch = N // (2 * ncores)          # nodes per half
    xb = np.asarray(x_batch)

    counts = np.bincount(xb, minlength=NSEG).astype(np.float64)

    halves = []                      # (src, seg_of_sb) per (core, half)
    n_sb = 0
    for c in range(ncores):
        for h in range(2):
            lo = (2 * c + h) * ch
            src, seg_of_sb = _pad_runs(xb[lo:lo + ch], lo, ch)
            halves.append((src, seg_of_sb))
            n_sb = max(n_sb, len(seg_of_sb))

    cols = n_sb * SB
    padcount = np.zeros(NSEG, dtype=np.float64)
    xts = []
    seg_a, seg_d = [], []
    for c in range(ncores):
        xt = np.zeros((128, cols), dtype=ml_dtypes.bfloat16)
        for h in range(2):
            src, seg_of_sb = halves[2 * c + h]
            if len(src) < cols:
                src = np.concatenate([src, np.full(cols - len(src), -1, np.int64)])
            mask = src >= 0
            gath = np.zeros((cols, 64), dtype=np.float32)
            gath[mask] = x[src[mask]]
            xt[64 * h:64 * h + 64, :] = gath.T.astype(ml_dtypes.bfloat16)
            if seg_of_sb:
                seg_arr = np.array(seg_of_sb, dtype=np.int64)
                real = mask[:len(seg_arr) * SB].reshape(-1, SB).sum(axis=1)
                np.add.at(padcount, seg_arr, SB - real)
            (seg_a if h == 0 else seg_d).append(seg_of_sb)
        xts.append(xt)

    meta = dict(n_sb=n_sb, cols=cols, counts=counts, padcount=padcount,
                seg_a=seg_a, seg_d=seg_d, ncores=ncores)
    return xts, meta


def _build_phase1(n_sb, cols, ncores=NCORES):
    nc = bacc.Bacc("TRN2", target_bir_lowering=False, debug=False,
                   num_devices=ncores)
    xt_d = nc.dram_tensor("xt", [128, cols], BF16, kind="ExternalInput").ap()
    wza_d = nc.dram_tensor("wza", [128, 128], BF16, kind="ExternalInput").ap()
    wzb_d = nc.dram_tensor("wzb", [128, 128], BF16, kind="ExternalInput").ap()
    b1_d = nc.dram_tensor("b1", [128, 1], F32, kind="ExternalInput").ap()
    nb1_d = nc.dram_tensor("nb1", [128, 1], F32, kind="ExternalInput").ap()
    sa_d = nc.dram_tensor("s_act", [128, n_sb], F32, kind="ExternalOutput").ap()
    sd_d = nc.dram_tensor("s_dve", [128, n_sb], F32, kind="ExternalOutput").ap()

    CH = SB_PER_CHUNK * SB

    with tile.TileContext(nc) as tc:
        with tc.tile_pool(name="const", bufs=1) as cpool, \
             tc.tile_pool(name="xin", bufs=3) as xpool, \
             tc.tile_pool(name="tr", bufs=1) as trpool, \
             tc.tile_pool(name="ps", bufs=2, space="PSUM") as pspool:

            wza = cpool.tile([128, 128], BF16)
            nc.sync.dma_start(wza[:], wza_d[:])
            wzb = cpool.tile([128, 128], BF16)
            nc.sync.dma_start(wzb[:], wzb_d[:])
            b1t = cpool.tile([128, 1], F32)
            nc.sync.dma_start(b1t[:], b1_d[:])
            nb1t = cpool.tile([128, 1], F32)
            nc.sync.dma_start(nb1t[:], nb1_d[:])
            S_a = cpool.tile([128, n_sb], F32)
            nc.vector.memset(S_a[:], 0.0)
            S_d = cpool.tile([128, n_sb], F32)
            nc.vector.memset(S_d[:], 0.0)

            xtile = None
            for sb in range(n_sb):
                if sb % SB_PER_CHUNK == 0:
                    w = min(CH, cols - sb * SB)
                    xtile = xpool.tile([128, CH], BF16, tag="x")
                    nc.sync.dma_start(xtile[:, :w], xt_d[:, sb * SB:sb * SB + w])
                base = (sb % SB_PER_CHUNK) * SB
                psa = pspool.tile([128, SB], F32, tag="psa")
                psb = pspool.tile([128, SB], F32, tag="psb")
                for t in range(SB // TILE):
                    off = base + t * TILE
                    nc.tensor.matmul(
                        psa[:, t * TILE:t * TILE + TILE], lhsT=wza[:],
                        rhs=xtile[:, off:off + TILE], start=True, stop=True)
                for t in range(SB // TILE):
                    off = base + t * TILE
                    nc.tensor.matmul(
                        psb[:, t * TILE:t * TILE + TILE], lhsT=wzb[:],
                        rhs=xtile[:, off:off + TILE], start=True, stop=True)
                trash_a = trpool.tile([128, SB], BF16, tag="ta")
                nc.scalar.activation(
                    out=trash_a[:], in_=psa[:],
                    func=mybir.ActivationFunctionType.Relu,
                    bias=b1t[:, 0:1],
                    accum_out=S_a[:, sb:sb + 1])
                # accum_out = add-reduce of max(psum, -b1)
                #           = sum(relu(psum + b1)) - SB*b1  (host adds it back)
                trash_d = trpool.tile([128, SB], BF16, tag="td")
                nc.vector.tensor_scalar(
                    out=trash_d[:], in0=psb[:],
                    scalar1=nb1t[:, 0:1], scalar2=0.0,
                    op0=mybir.AluOpType.max, op1=mybir.AluOpType.add,
                    accum_out=S_d[:, sb:sb + 1])

            nc.sync.dma_start(sa_d[:], S_a[:])
            nc.sync.dma_start(sd_d[:], S_d[:])

    nc.compile()
    return nc


def _build_phase2():
    nc = bacc.Bacc("TRN2", target_bir_lowering=False, debug=False, num_devices=1)
    mean_d = nc.dram_tensor("mean", [128, NSEG], BF16, kind="ExternalInput").ap()
    w2_d = nc.dram_tensor("w2", [128, 128], BF16, kind="ExternalInput").ap()
    w3_d = nc.dram_tensor("w3", [128, 128], BF16, kind="ExternalInput").ap()
    w4_d = nc.dram_tensor("w4", [128, ODIM], BF16, kind="ExternalInput").ap()
    b2_d = nc.dram_tensor("b2", [128, 1], F32, kind="ExternalInput").ap()
    b3_d = nc.dram_tensor("b3", [128, 1], F32, kind="ExternalInput").ap()
    b4_d = nc.dram_tensor("b4", [ODIM, 1], F32, kind="ExternalInput").ap()
    out_d = nc.dram_tensor("out_t", [ODIM, NSEG], F32, kind="ExternalOutput").ap()

    with tile.TileContext(nc) as tc:
        with tc.tile_pool(name="sb", bufs=1) as pool, \
             tc.tile_pool(name="ps", bufs=2, space="PSUM") as psp:
            mean = pool.tile([128, NSEG], BF16)
            nc.sync.dma_start(mean[:], mean_d[:])
            w2 = pool.tile([128, 128], BF16)
            nc.sync.dma_start(w2[:], w2_d[:])
            w3 = pool.tile([128, 128], BF16)
            nc.sync.dma_start(w3[:], w3_d[:])
            w4 = pool.tile([128, ODIM], BF16)
            nc.sync.dma_start(w4[:], w4_d[:])
            b2 = pool.tile([128, 1], F32)
            nc.sync.dma_start(b2[:], b2_d[:])
            b3 = pool.tile([128, 1], F32)
            nc.sync.dma_start(b3[:], b3_d[:])
            b4 = pool.tile([ODIM, 1], F32)
            nc.sync.dma_start(b4[:], b4_d[:])

            hid = pool.tile([128, NSEG], BF16)
            t3 = pool.tile([128, NSEG], BF16)
            ot = pool.tile([ODIM, NSEG], F32)
            for j in range(NSEG // 512):
                sl = slice(512 * j, 512 * j + 512)
                p2 = psp.tile([128, 512], F32, tag="p")
                nc.tensor.matmul(p2[:], lhsT=w2[:], rhs=mean[:, sl],
                                 start=True, stop=True)
                nc.scalar.activation(out=hid[:, sl], in_=p2[:],
                                     func=mybir.ActivationFunctionType.Identity,
                                     bias=b2[:, 0:1])
            for j in range(NSEG // 512):
                sl = slice(512 * j, 512 * j + 512)
                p3 = psp.tile([128, 512], F32, tag="p")
                nc.tensor.matmul(p3[:], lhsT=w3[:], rhs=hid[:, sl],
                                 start=True, stop=True)
                nc.scalar.activation(out=t3[:, sl], in_=p3[:],
                                     func=mybir.ActivationFunctionType.Relu,
                                     bias=b3[:, 0:1])
            for j in range(NSEG // 512):
                sl = slice(512 * j, 512 * j + 512)
                p4f = psp.tile([128, 512], F32, tag="p")
                p4 = p4f[:ODIM, :]
                nc.tensor.matmul(p4, lhsT=w4[:], rhs=t3[:, sl],
                                 start=True, stop=True)
                nc.scalar.activation(out=ot[:, sl], in_=p4,
                                     func=mybir.ActivationFunctionType.Identity,
                                     bias=b4[:, 0:1])
            nc.sync.dma_start(out_d[:], ot[:])
    nc.compile()
    return nc


def run(inputs, ncores=NCORES, trace=False):
    x = np.asarray(inputs["x"], dtype=np.float32)
    xb = np.asarray(inputs["x_batch"])
    W1 = np.asarray(inputs["W1"], dtype=np.float32)
    b1 = np.asarray(inputs["b1"], dtype=np.float32)

    xts, meta = _host_prep(x, xb, ncores=ncores)
    n_sb, cols = meta["n_sb"], meta["cols"]

    wza = np.zeros((128, 128), dtype=np.float32)
    wza[0:64, :] = W1
    wzb = np.zeros((128, 128), dtype=np.float32)
    wzb[64:128, :] = W1
    wza = wza.astype(ml_dtypes.bfloat16)
    wzb = wzb.astype(ml_dtypes.bfloat16)
    b1c = np.ascontiguousarray(b1, np.float32).reshape(128, 1)
    nb1c = np.ascontiguousarray(-b1, np.float32).reshape(128, 1)
    in_maps = [dict(xt=xts[c], wza=wza, wzb=wzb, b1=b1c, nb1=nb1c)
               for c in range(ncores)]

    nc1 = _build_phase1(n_sb, cols, ncores=ncores)
    res1 = run_bass_kernel_spmd(nc1, in_maps, core_ids=list(range(ncores)),
                                trace=trace)

    # host: route superblock partials to segments, 8-core combine.
    # Vector-path sums are sum(max(psum,-b1)) = sum(relu(psum+b1)) - SB*b1.
    b1f = b1.astype(np.float64)
    gsums = np.zeros((NSEG, 128), dtype=np.float64)
    for c in range(ncores):
        Sa = res1.results[c]["s_act"].astype(np.float64)   # [128, n_sb]
        Sd = res1.results[c]["s_dve"].astype(np.float64)
        seg = np.array(meta["seg_a"][c], dtype=np.int64)
        if len(seg):
            np.add.at(gsums, seg, Sa.T[:len(seg)])
        seg = np.array(meta["seg_d"][c], dtype=np.int64)
        if len(seg):
            np.add.at(gsums, seg, Sd.T[:len(seg)] + SB * b1f[None, :])
    # remove the relu(b1) contribution of zero-pad columns
    gsums -= np.maximum(b1, 0.0)[None, :].astype(np.float64) * meta["padcount"][:, None]

    counts = meta["counts"]
    mean = gsums / np.maximum(counts, 1.0)[:, None]                  # [NSEG,128]

    p2_ins = [dict(
        mean=np.ascontiguousarray(mean.T.astype(ml_dtypes.bfloat16)),
        w2=np.ascontiguousarray(inputs["W2"], np.float32).astype(ml_dtypes.bfloat16),
        w3=np.ascontiguousarray(inputs["W3"], np.float32).astype(ml_dtypes.bfloat16),
        w4=np.ascontiguousarray(inputs["W4"], np.float32).astype(ml_dtypes.bfloat16),
        b2=np.ascontiguousarray(inputs["b2"], np.float32).reshape(128, 1),
        b3=np.ascontiguousarray(inputs["b3"], np.float32).reshape(128, 1),
        b4=np.ascontiguousarray(inputs["b4"], np.float32).reshape(ODIM, 1),
    )]
    nc2 = _build_phase2()
    res2 = run_bass_kernel_spmd(nc2, p2_ins, core_ids=[0], trace=trace)
    out = np.ascontiguousarray(res2.results[0]["out_t"].T).astype(np.float32)

    # segments with no nodes: reference's hid is 0 (not b2), so
    # out = relu(b3) @ W4 + b4 exactly
    empty = counts == 0
    if empty.any():
        row = (np.maximum(np.asarray(inputs["b3"], np.float64), 0.0)
               @ np.asarray(inputs["W4"], np.float64)
               + np.asarray(inputs["b4"], np.float64))
        out[empty] = row.astype(np.float32)
    return out, res1, res2


def kernel(**inputs):
    inputs = {k: np.asarray(v) for k, v in inputs.items()}
    out, _, _ = run(inputs)
    return out


if __name__ == "__main__":
    rng = np.random.default_rng(0)
    N, D, HN, B = 8 * 32 * SB, 64, 128, 64
    x = rng.standard_normal((N, D), dtype=np.float32)
    xb = np.sort(rng.integers(0, B, N).astype(np.int32))
    W1 = (rng.standard_normal((D, HN)) / 8).astype(np.float32)
    W2 = (rng.standard_normal((HN, HN)) / 11.3).astype(np.float32)
    W3 = (rng.standard_normal((HN, HN)) / 11.3).astype(np.float32)
    W4 = (rng.standard_normal((HN, ODIM)) / 11.3).astype(np.float32)
    b1 = rng.standard_normal(HN).astype(np.float32) * 0.1
    b2 = rng.standard_normal(HN).astype(np.float32) * 0.1
    b3 = rng.standard_normal(HN).astype(np.float32) * 0.1
    b4 = rng.standard_normal(ODIM).astype(np.float32) * 0.1
    ins = dict(x=x, x_batch=xb, W1=W1, b1=b1, W2=W2, b2=b2, W3=W3, b3=b3,
               W4=W4, b4=b4)
    out = kernel(**ins)

    h = np.maximum(x @ W1 + b1, 0) @ W2 + b2
    sums = np.zeros((1024, HN), dtype=np.float64)
    np.add.at(sums, xb, h.astype(np.float64))
    cnt = np.bincount(xb, minlength=1024).astype(np.float64)
    mean = sums / np.maximum(cnt, 1)[:, None]
    ref = (np.maximum(mean @ W3 + b3, 0) @ W4 + b4).astype(np.float32)
    num = np.linalg.norm(out - ref)
    den = np.linalg.norm(ref)
    print("Relative error:", num / den)


# revision 12
# speedup vs baseline: 2.0036x; 1.0415x over previous
"""DeepSet segment-reduce kernel for 8 Trainium2 NeuronCores.

Math (equivalent to the reference, using linearity of segment_sum):
    r      = relu(x @ W1 + b1)                      # per-node, on device
    sums_r = segment_sum(r)                         # [B, HID]
    mean_r = sums_r / max(counts, 1)                # counts via host bincount
    hid    = mean_r @ W2 + b2                       # tiny tail, on device
    out    = relu(hid @ W3 + b3) @ W4 + b4          # tiny tail, on device

Phase 1 (8 cores, data-parallel over nodes): each core's ~N/8 nodes are
split into two contiguous halves packed on SBUF partition halves (features
of half A in partitions 0..63, half B in 64..127, bf16), so DMA runs at
full 128-partition width.  Each half's segment runs are zero-padded to
multiples of 1024 columns, so every 1024-column superblock belongs to one
segment.  Matmuls use K=128 with zero-padded weights wzA=[[W1],[0]],
wzB=[[0],[W1]] — full-array matmuls keep the PE's HAM activity monitor
un-throttled at 2.4 GHz (K=64 matmuls leave it stuck at 1.2 GHz), and both
weight tiles live at PE tile position (0,0) (bf16 LDWEIGHTS at row tile 64
is broken in hardware).  Per superblock and half: 2 matmuls fill a 2-bank
PSUM tile, then ONE fused relu+bias+sum produces the [128,1] partial:
half A on the Scalar engine (ACT Relu, bias, accum_out), half B on the
Vector engine (tensor_scalar max(-b1)/add-reduce; the sum is off by
SB*b1, restored on the host).  With 4 PSUM tiles in flight both reduce
engines run continuously while the PE refills the other tiles.

The host routes superblock partials to segments, removes the pad columns'
relu(b1) contribution, applies the mean, and a second tiny NEFF runs the
rho MLP in bf16.  Segments with zero nodes are fixed up on the host
(reference gives relu(b3) @ W4 + b4 there).
"""

import os
import sys

for _p in ("/opt/trn_rl_repo",):
    if os.path.isdir(_p) and _p not in sys.path:
        sys.path.append(_p)

import numpy as np
import ml_dtypes

import concourse.bass as bass
import concourse.tile as tile
from concourse import bacc, mybir
from concourse.bass_utils import run_bass_kernel_spmd

F32 = mybir.dt.float32
BF16 = mybir.dt.bfloat16

NCORES = 8
TILE = 512
SB = 1024            # superblock columns (2 PSUM banks; 4 tiles in flight)
SB_PER_CHUNK = 8     # superblocks per DMA chunk
NSEG = 1024
ODIM = 16


def _pad_runs(ids, lo, ch):
    """Segment runs of a sorted id slice, padded to SB multiples.
    Returns (src indices with -1 pads, seg id per superblock)."""
    uniq, starts = np.unique(ids, return_index=True)
    ends = np.append(starts[1:], ch)
    seg_of_sb = []
    src_parts = []
    for k in range(len(uniq)):
        L = int(ends[k] - starts[k])
        T = -(-L // SB)
        arr = np.full(T * SB, -1, dtype=np.int64)
        arr[:L] = lo + starts[k] + np.arange(L)
        src_parts.append(arr)
        seg_of_sb += [int(uniq[k])] * T
    src = np.concatenate(src_parts) if src_parts else np.empty(0, np.int64)
    return src, seg_of_sb


def _host_prep(x, x_batch, ncores=NCORES):
    N = x.shape[0]
    assert N % (2 * ncores) == 0
    ch = N // (2 * ncores)          # nodes per half
    xb = np.asarray(x_batch)

    counts = np.bincount(xb, minlength=NSEG).astype(np.float64)

    halves = []                      # (src, seg_of_sb) per (core, half)
    n_sb = 0
    for c in range(ncores):
        for h in range(2):
            lo = (2 * c + h) * ch
            src, seg_of_sb = _pad_runs(xb[lo:lo + ch], lo, ch)
            halves.append((src, seg_of_sb))
            n_sb = max(n_sb, len(seg_of_sb))

    cols = n_sb * SB
    padcount = np.zeros(NSEG, dtype=np.float64)
    xts = []
    seg_a, seg_d = [], []
    for c in range(ncores):
        xt = np.zeros((128, cols), dtype=ml_dtypes.bfloat16)
        for h in range(2):
            src, seg_of_sb = halves[2 * c + h]
            if len(src) < cols:
                src = np.concatenate([src, np.full(cols - len(src), -1, np.int64)])
            mask = src >= 0
            gath = np.zeros((cols, 64), dtype=np.float32)
            gath[mask] = x[src[mask]]
            xt[64 * h:64 * h + 64, :] = gath.T.astype(ml_dtypes.bfloat16)
            if seg_of_sb:
                seg_arr = np.array(seg_of_sb, dtype=np.int64)
                real = mask[:len(seg_arr) * SB].reshape(-1, SB).sum(axis=1)
                np.add.at(padcount, seg_arr, SB - real)
            (seg_a if h == 0 else seg_d).append(seg_of_sb)
        xts.append(xt)

    meta = dict(n_sb=n_sb, cols=cols, counts=counts, padcount=padcount,
                seg_a=seg_a, seg_d=seg_d, ncores=ncores)
    return xts, meta


def _build_phase1(n_sb, cols, ncores=NCORES):
    nc = bacc.Bacc("TRN2", target_bir_lowering=False, debug=False,
                   num_devices=ncores)
    xt_d = nc.dram_tensor("xt", [128, cols], BF16, kind="ExternalInput").ap()
    wza_d = nc.dram_tensor("wza", [128, 128], BF16, kind="ExternalInput").ap()
    wzb_d = nc.dram_tensor("wzb", [128, 128], BF16, kind="ExternalInput").ap()
    b1_d = nc.dram_tensor("b1", [128, 1], F32, kind="ExternalInput").ap()
    nb1_d = nc.dram_tensor("nb1", [128, 1], F32, kind="ExternalInput").ap()
    sa_d = nc.dram_tensor("s_act", [128, n_sb], F32, kind="ExternalOutput").ap()
    sd_d = nc.dram_tensor("s_dve", [128, n_sb], F32, kind="ExternalOutput").ap()

    CH = SB_PER_CHUNK * SB

    with tile.TileContext(nc) as tc:
        with tc.tile_pool(name="const", bufs=1) as cpool, \
             tc.tile_pool(name="xin", bufs=3) as xpool, \
             tc.tile_pool(name="tr", bufs=1) as trpool, \
             tc.tile_pool(name="ps", bufs=2, space="PSUM") as pspool:

            wza = cpool.tile([128, 128], BF16)
            nc.sync.dma_start(wza[:], wza_d[:])
            wzb = cpool.tile([128, 128], BF16)
            nc.sync.dma_start(wzb[:], wzb_d[:])
            b1t = cpool.tile([128, 1], F32)
            nc.sync.dma_start(b1t[:], b1_d[:])
            nb1t = cpool.tile([128, 1], F32)
            nc.sync.dma_start(nb1t[:], nb1_d[:])
            S_a = cpool.tile([128, n_sb], F32)
            nc.vector.memset(S_a[:], 0.0)
            S_d = cpool.tile([128, n_sb], F32)
            nc.vector.memset(S_d[:], 0.0)

            xtile = None
            for sb in range(n_sb):
                if sb % SB_PER_CHUNK == 0:
                    # one SBUF chunk, filled by per-superblock DMAs so the
                    # first matmuls start after ~256 KB instead of ~2 MB
                    xtile = xpool.tile([128, CH], BF16, tag="x")
                    for j in range(min(SB_PER_CHUNK, n_sb - sb)):
                        lo = (sb + j) * SB
                        nc.sync.dma_start(xtile[:, j * SB:(j + 1) * SB],
                                          xt_d[:, lo:lo + SB])
                base = (sb % SB_PER_CHUNK) * SB
                psa = pspool.tile([128, SB], F32, tag="psa")
                psb = pspool.tile([128, SB], F32, tag="psb")
                for t in range(SB // TILE):
                    off = base + t * TILE
                    nc.tensor.matmul(
                        psa[:, t * TILE:t * TILE + TILE], lhsT=wza[:],
                        rhs=xtile[:, off:off + TILE], start=True, stop=True)
                for t in range(SB // TILE):
                    off = base + t * TILE
                    nc.tensor.matmul(
                        psb[:, t * TILE:t * TILE + TILE], lhsT=wzb[:],
                        rhs=xtile[:, off:off + TILE], start=True, stop=True)
                trash_a = trpool.tile([128, SB], BF16, tag="ta")
                nc.scalar.activation(
                    out=trash_a[:], in_=psa[:],
                    func=mybir.ActivationFunctionType.Relu,
                    bias=b1t[:, 0:1],
                    accum_out=S_a[:, sb:sb + 1])
                # accum_out = add-reduce of max(psum, -b1)
                #           = sum(relu(psum + b1)) - SB*b1  (host adds it back)
                trash_d = trpool.tile([128, SB], BF16, tag="td")
                nc.vector.tensor_scalar(
                    out=trash_d[:], in0=psb[:],
                    scalar1=nb1t[:, 0:1], scalar2=0.0,
                    op0=mybir.AluOpType.max, op1=mybir.AluOpType.add,
                    accum_out=S_d[:, sb:sb + 1])

            nc.sync.dma_start(sa_d[:], S_a[:])
            nc.sync.dma_start(sd_d[:], S_d[:])

    nc.compile()
    return nc


def _build_phase2():
    nc = bacc.Bacc("TRN2", target_bir_lowering=False, debug=False, num_devices=1)
    mean_d = nc.dram_tensor("mean", [128, NSEG], BF16, kind="ExternalInput").ap()
    w2_d = nc.dram_tensor("w2", [128, 128], BF16, kind="ExternalInput").ap()
    w3_d = nc.dram_tensor("w3", [128, 128], BF16, kind="ExternalInput").ap()
    w4_d = nc.dram_tensor("w4", [128, ODIM], BF16, kind="ExternalInput").ap()
    b2_d = nc.dram_tensor("b2", [128, 1], F32, kind="ExternalInput").ap()
    b3_d = nc.dram_tensor("b3", [128, 1], F32, kind="ExternalInput").ap()
    b4_d = nc.dram_tensor("b4", [ODIM, 1], F32, kind="ExternalInput").ap()
    out_d = nc.dram_tensor("out_t", [ODIM, NSEG], F32, kind="ExternalOutput").ap()

    with tile.TileContext(nc) as tc:
        with tc.tile_pool(name="sb", bufs=1) as pool, \
             tc.tile_pool(name="ps", bufs=2, space="PSUM") as psp:
            mean = pool.tile([128, NSEG], BF16)
            nc.sync.dma_start(mean[:], mean_d[:])
            w2 = pool.tile([128, 128], BF16)
            nc.sync.dma_start(w2[:], w2_d[:])
            w3 = pool.tile([128, 128], BF16)
            nc.sync.dma_start(w3[:], w3_d[:])
            w4 = pool.tile([128, ODIM], BF16)
            nc.sync.dma_start(w4[:], w4_d[:])
            b2 = pool.tile([128, 1], F32)
            nc.sync.dma_start(b2[:], b2_d[:])
            b3 = pool.tile([128, 1], F32)
            nc.sync.dma_start(b3[:], b3_d[:])
            b4 = pool.tile([ODIM, 1], F32)
            nc.sync.dma_start(b4[:], b4_d[:])

            hid = pool.tile([128, NSEG], BF16)
            t3 = pool.tile([128, NSEG], BF16)
            ot = pool.tile([ODIM, NSEG], F32)
            for j in range(NSEG // 512):
                sl = slice(512 * j, 512 * j + 512)
                p2 = psp.tile([128, 512], F32, tag="p")
                nc.tensor.matmul(p2[:], lhsT=w2[:], rhs=mean[:, sl],
                                 start=True, stop=True)
                nc.scalar.activation(out=hid[:, sl], in_=p2[:],
                                     func=mybir.ActivationFunctionType.Identity,
                                     bias=b2[:, 0:1])
            for j in range(NSEG // 512):
                sl = slice(512 * j, 512 * j + 512)
                p3 = psp.tile([128, 512], F32, tag="p")
                nc.tensor.matmul(p3[:], lhsT=w3[:], rhs=hid[:, sl],
                                 start=True, stop=True)
                nc.scalar.activation(out=t3[:, sl], in_=p3[:],
                                     func=mybir.ActivationFunctionType.Relu,
                                     bias=b3[:, 0:1])
            for j in range(NSEG // 512):
                sl = slice(512 * j, 512 * j + 512)
                p4f = psp.tile([128, 512], F32, tag="p")
                p4 = p4f[:ODIM, :]
                nc.tensor.matmul(p4, lhsT=w4[:], rhs=t3[:, sl],
                                 start=True, stop=True)
                nc.scalar.activation(out=ot[:, sl], in_=p4,
                                     func=mybir.ActivationFunctionType.Identity,
                                     bias=b4[:, 0:1])
            nc.sync.dma_start(out_d[:], ot[:])
    nc.compile()
    return nc


def run(inputs, ncores=NCORES, trace=False):
    x = np.asarray(inputs["x"], dtype=np.float32)
    xb = np.asarray(inputs["x_batch"])
    W1 = np.asarray(inputs["W1"], dtype=np.float32)
    b1 = np.asarray(inputs["b1"], dtype=np.float32)

    xts, meta = _host_prep(x, xb, ncores=ncores)
    n_sb, cols = meta["n_sb"], meta["cols"]

    wza = np.zeros((128, 128), dtype=np.float32)
    wza[0:64, :] = W1
    wzb = np.zeros((128, 128), dtype=np.float32)
    wzb[64:128, :] = W1
    wza = wza.astype(ml_dtypes.bfloat16)
    wzb = wzb.astype(ml_dtypes.bfloat16)
    b1c = np.ascontiguousarray(b1, np.float32).reshape(128, 1)
    nb1c = np.ascontiguousarray(-b1, np.float32).reshape(128, 1)
    in_maps = [dict(xt=xts[c], wza=wza, wzb=wzb, b1=b1c, nb1=nb1c)
               for c in range(ncores)]

    nc1 = _build_phase1(n_sb, cols, ncores=ncores)
    res1 = run_bass_kernel_spmd(nc1, in_maps, core_ids=list(range(ncores)),
                                trace=trace)

    # host: route superblock partials to segments, 8-core combine.
    # Vector-path sums are sum(max(psum,-b1)) = sum(relu(psum+b1)) - SB*b1.
    b1f = b1.astype(np.float64)
    gsums = np.zeros((NSEG, 128), dtype=np.float64)
    for c in range(ncores):
        Sa = res1.results[c]["s_act"].astype(np.float64)   # [128, n_sb]
        Sd = res1.results[c]["s_dve"].astype(np.float64)
        seg = np.array(meta["seg_a"][c], dtype=np.int64)
        if len(seg):
            np.add.at(gsums, seg, Sa.T[:len(seg)])
        seg = np.array(meta["seg_d"][c], dtype=np.int64)
        if len(seg):
            np.add.at(gsums, seg, Sd.T[:len(seg)] + SB * b1f[None, :])
    # remove the relu(b1) contribution of zero-pad columns
    gsums -= np.maximum(b1, 0.0)[None, :].astype(np.float64) * meta["padcount"][:, None]

    counts = meta["counts"]
    mean = gsums / np.maximum(counts, 1.0)[:, None]                  # [NSEG,128]

    p2_ins = [dict(
        mean=np.ascontiguousarray(mean.T.astype(ml_dtypes.bfloat16)),
        w2=np.ascontiguousarray(inputs["W2"], np.float32).astype(ml_dtypes.bfloat16),
        w3=np.ascontiguousarray(inputs["W3"], np.float32).astype(ml_dtypes.bfloat16),
        w4=np.ascontiguousarray(inputs["W4"], np.float32).astype(ml_dtypes.bfloat16),
        b2=np.ascontiguousarray(inputs["b2"], np.float32).reshape(128, 1),
        b3=np.ascontiguousarray(inputs["b3"], np.float32).reshape(128, 1),
        b4=np.ascontiguousarray(inputs["b4"], np.float32).reshape(ODIM, 1),
    )]
    nc2 = _build_phase2()
    res2 = run_bass_kernel_spmd(nc2, p2_ins, core_ids=[0], trace=trace)
    out = np.ascontiguousarray(res2.results[0]["out_t"].T).astype(np.float32)

    # segments with no nodes: reference's hid is 0 (not b2), so
    # out = relu(b3) @ W4 + b4 exactly
    empty = counts == 0
    if empty.any():
        row = (np.maximum(np.asarray(inputs["b3"], np.float64), 0.0)
               @ np.asarray(inputs["W4"], np.float64)
               + np.asarray(inputs["b4"], np.float64))
        out[empty] = row.astype(np.float32)
    return out, res1, res2


def kernel(**inputs):
    inputs = {k: np.asarray(v) for k, v in inputs.items()}
    out, _, _ = run(inputs)
    return out


if __name__ == "__main__":
    rng = np.random.default_rng(0)
    N, D, HN, B = 8 * 32 * SB, 64, 128, 64
    x = rng.standard_normal((N, D), dtype=np.float32)
    xb = np.sort(rng.integers(0, B, N).astype(np.int32))
    W1 = (rng.standard_normal((D, HN)) / 8).astype(np.float32)
    W2 = (rng.standard_normal((HN, HN)) / 11.3).astype(np.float32)
    W3 = (rng.standard_normal((HN, HN)) / 11.3).astype(np.float32)
    W4 = (rng.standard_normal((HN, ODIM)) / 11.3).astype(np.float32)
    b1 = rng.standard_normal(HN).astype(np.float32) * 0.1
    b2 = rng.standard_normal(HN).astype(np.float32) * 0.1
    b3 = rng.standard_normal(HN).astype(np.float32) * 0.1
    b4 = rng.standard_normal(ODIM).astype(np.float32) * 0.1
    ins = dict(x=x, x_batch=xb, W1=W1, b1=b1, W2=W2, b2=b2, W3=W3, b3=b3,
               W4=W4, b4=b4)
    out = kernel(**ins)

    h = np.maximum(x @ W1 + b1, 0) @ W2 + b2
    sums = np.zeros((1024, HN), dtype=np.float64)
    np.add.at(sums, xb, h.astype(np.float64))
    cnt = np.bincount(xb, minlength=1024).astype(np.float64)
    mean = sums / np.maximum(cnt, 1)[:, None]
    ref = (np.maximum(mean @ W3 + b3, 0) @ W4 + b4).astype(np.float32)
    num = np.linalg.norm(out - ref)
    den = np.linalg.norm(ref)
    print("Relative error:", num / den)
